# revision 33
# baseline (speedup 1.0000x reference)
"""Trainium2 Bass kernel for nn_Attention_21406117003362.

Computes, for features [B,N,2H], query [B,1,H], Wf [H,2H], Wq [H,H], v [1,1,H]:
  pf     = features @ Wf.T                      [B,N,H]
  pq     = query @ Wq.T                         [B,1,H]
  u      = sum_h v[h] * tanh(pf + pq)           [B,1,N]
  logits = 10 * tanh(u)                         [B,1,N]
returns (pf, logits).

Sharding: data-parallel over B across 8 NeuronCores (2 batches/core),
Wf/Wq/v replicated.

Per-core schedule (single SPMD program):
  - Wf/Wq are PE-transposed once into [f_part, h_free] layout; pq and v are
    broadcast across partitions via tiny K=1 matmuls.
  - Main loop over 64 n-chunks of 128 rows, software-pipelined by one chunk:
    the PE stream interleaves chunk i+1's feature-tile transposes (fp32,
    exact) with chunk i's 8 accumulating float32r (tf32) matmuls so weight
    loads hide under matmul streaming. psum [n128, h512] is the natural
    output layout and is DMA'd straight to DRAM.
  - DVE adds pq; ACT applies tanh and then folds the v-weighted reduction
    over h into one Copy-with-scale + accum_out pass.
  - Per batch, the 32 u-columns are PE-transposed and 10*tanh applied.
"""

import numpy as np

import bass_rust
import concourse.bass as bass
import concourse.mybir as mybir
import concourse.tile as tile
from concourse.bass_utils import run_bass_kernel_spmd
from concourse.masks import make_identity

B, N, H = 16, 4096, 512
F = 2 * H  # 1024
NCORES = 8
BL = B // NCORES  # batches per core = 2
P = 128
NCHUNKS = N // P  # 32 chunks per batch
FCHUNKS = F // P  # 8
TANH_CLIP = 10.0

f32 = mybir.dt.float32
f32r = mybir.dt.float32r

_MAX_WAITS = 1


def _split_excess_waits(nc):
    """This walrus build only encodes 1 sem-wait per instruction; hoist
    extras onto preceding NOPs on the same engine."""
    for bb in nc.m.functions[0].blocks:
        new, changed = [], False
        for inst in bb.instructions:
            si = inst.sync_info
            if si and si.on_wait and len(si.on_wait) > _MAX_WAITS:
                w, k = list(si.on_wait), 0
                while len(w) > _MAX_WAITS:
                    chunk, w = w[:_MAX_WAITS], w[_MAX_WAITS:]
                    new.append(mybir.InstNoOp(
                        name=f"{inst.name}-wsplit{k}", engine=inst.engine,
                        ins=[], outs=[],
                        sync_info=bass_rust.SyncInfo(on_wait=chunk, on_update=[])))
                    k += 1
                inst.sync_info = bass_rust.SyncInfo(on_wait=w, on_update=si.on_update)
                changed = True
            new.append(inst)
        if changed:
            bb.instructions = new


def build_program(split_waits=True):
    nc = bass.Bass("TRN2", target_bir_lowering=False, debug=False)

    feat_d = nc.dram_tensor("features", [BL, N, F], f32, kind="ExternalInput").ap()
    query_d = nc.dram_tensor("query", [BL, 1, H], f32, kind="ExternalInput").ap()
    wf_d = nc.dram_tensor("Wf", [H, F], f32, kind="ExternalInput").ap()
    wq_d = nc.dram_tensor("Wq", [H, H], f32, kind="ExternalInput").ap()
    v_d = nc.dram_tensor("v", [1, 1, H], f32, kind="ExternalInput").ap()

    pf_d = nc.dram_tensor("pf", [BL, N, H], f32, kind="ExternalOutput").ap()
    logits_d = nc.dram_tensor("logits", [BL, 1, N], f32, kind="ExternalOutput").ap()

    HCHUNKS = H // P  # 4
    TOTAL = BL * NCHUNKS  # 64 chunks

    with tile.TileContext(nc) as tc:
        with (
            tc.tile_pool(name="const", bufs=1) as const,
            tc.tile_pool(name="setup", bufs=1) as setup,
            tc.tile_pool(name="feat", bufs=8) as featp,
            tc.tile_pool(name="featT", bufs=6) as featTp,
            tc.tile_pool(name="eltw", bufs=6) as eltw,
            tc.tile_pool(name="ps_t", bufs=4, space="PSUM") as ps_t,
            tc.tile_pool(name="ps_pf", bufs=4, space="PSUM") as ps_pf,
        ):
            # ---------------- warmup + early loads ----------------
            # keep the PE busy from t=0 so the HAM clock-gate reaches 8/8
            # before the real work, and queue feature loads ahead of the
            # weight-setup DMAs on the SP queue
            ident = const.tile([P, P], f32)
            warm = const.tile([P, P], f32)
            nc.vector.memset(warm[:], 0.0)
            make_identity(nc, ident[:])

            ones_row = const.tile([1, P], f32)
            nc.vector.memset(ones_row[:], 1.0)

            HCHUNKS = H // P  # 4
            GCH = H // P  # 4

            ftiles = [None] * TOTAL
            featTs = [None] * TOTAL
            ppfs = [None] * TOTAL

            def load(j):
                b, i = divmod(j, NCHUNKS)
                t = featp.tile([P, F], f32, name="ftile")
                nc.sync.dma_start(t[:], feat_d[b, i * P:(i + 1) * P, :])
                ftiles[j] = t

            load(0)
            load(1)
            wf_sb = setup.tile([P, HCHUNKS, F], f32)
            nc.sync.dma_start(wf_sb[:], wf_d.rearrange("(ho p) f -> p ho f", p=P))
            load(2)
            load(3)

            ps_warm = ps_t.tile([P, 512], f32, tag="pst", name="ps_warm")
            for _ in range(30):
                nc.tensor.matmul(ps_warm[:, :P], warm[:], warm[:],
                                 start=True, stop=True)

            # ---------------- setup ----------------
            load(4)
            load(5)
            wq_sb = setup.tile([P, GCH, H], f32)
            nc.sync.dma_start(wq_sb[:], wq_d.rearrange("(ho p) g -> p ho g", p=P))
            q_sb = setup.tile([BL, H], f32)
            nc.sync.dma_start(q_sb[:], query_d[:, 0, :])
            v_row = setup.tile([1, H], f32)
            nc.sync.dma_start(v_row[:], v_d[0, :, :])
            load(6)
            load(7)

            def transposes(j, half):
                pst = ps_t.tile([P, 512], f32, tag="pst", name="pst")
                for q in range(4):
                    fc = half * 4 + q
                    nc.tensor.transpose(
                        pst[:, q * P:(q + 1) * P],
                        ftiles[j][:, fc * P:(fc + 1) * P],
                        ident[:])
                if featTs[j] is None:
                    featTs[j] = featTp.tile([P, FCHUNKS, P], mybir.dt.float16, name="featT")
                if half == 0:
                    nc.scalar.copy(featTs[j][:, :4, :], pst[:])
                else:
                    nc.vector.tensor_copy(out=featTs[j][:, 4:, :], in_=pst[:])

            # first feature-tile transposes run while Wf/Wq still stream in
            for j in range(2):
                transposes(j, 0)
                transposes(j, 1)

            wfT = const.tile([P, FCHUNKS, H], mybir.dt.float16)
            for fc in range(FCHUNKS):
                pst = ps_t.tile([P, 512], f32, tag="pst", name="pst")
                for ho in range(HCHUNKS):
                    nc.tensor.transpose(
                        pst[:, ho * P:(ho + 1) * P],
                        wf_sb[:, ho, fc * P:(fc + 1) * P], ident[:])
                nc.any.tensor_copy(out=wfT[:, fc, :], in_=pst[:])

            # Wq -> wqT [g_part, gchunk, h]
            wqT = setup.tile([P, GCH, H], f32)
            for gc in range(GCH):
                pst = ps_t.tile([P, 512], f32, tag="pst", name="pst")
                for ho in range(GCH):
                    nc.tensor.transpose(
                        pst[:, ho * P:(ho + 1) * P],
                        wq_sb[:, ho, gc * P:(gc + 1) * P], ident[:])
                nc.any.tensor_copy(out=wqT[:, gc, :], in_=pst[:])

            # query -> qT [g_part, gchunk, b] via PE transpose of [2, 128] rows
            qT = setup.tile([P, GCH, BL], f32)
            for gc in range(GCH):
                pst = ps_t.tile([P, 512], f32, tag="pst", name="pst")
                nc.tensor.transpose(
                    pst[:P, :BL], q_sb[:, gc * P:(gc + 1) * P],
                    ident[:BL, :BL])
                nc.any.tensor_copy(out=qT[:, gc, :], in_=pst[:P, :BL])

            # pq_row_b [1, h] = query[b] @ Wq.T
            pq_rows = []
            for b in range(BL):
                ps_pq = ps_t.tile([P, 512], f32, tag="pst", name="ps_pq")
                for gc in range(GCH):
                    nc.tensor.matmul(
                        ps_pq[:1, :H], qT[:, gc, b:b + 1],
                        wqT[:, gc, :],
                        start=(gc == 0), stop=(gc == GCH - 1))
                t = setup.tile([1, H], f32, tag=f"pq_row{b}")
                nc.any.tensor_copy(out=t[:], in_=ps_pq[:1, :H])
                pq_rows.append(t)

            # broadcast pq rows and v across 128 partitions via K=1 matmuls
            v_bc2 = const.tile([P, 2, H], f32)
            ps_b = ps_t.tile([P, 512], f32, tag="pst", name="ps_b")
            nc.tensor.matmul(ps_b[:, :H], ones_row[:], v_row[:],
                             start=True, stop=True)
            nc.any.tensor_copy(out=v_bc2[:, 0, :], in_=ps_b[:, :H])
            nc.any.tensor_copy(out=v_bc2[:, 1, :], in_=ps_b[:, :H])

            pq_bc = []
            for b in range(BL):
                ps_b2 = ps_t.tile([P, 512], f32, tag="pst", name="ps_b2")
                nc.tensor.matmul(ps_b2[:, :H], ones_row[:], pq_rows[b][:],
                                 start=True, stop=True)
                t = const.tile([P, H], f32, tag=f"pq_bc{b}")
                nc.any.tensor_copy(out=t[:], in_=ps_b2[:, :H])
                pq_bc.append(t)

            u_cols = []
            for b in range(BL):
                uc = const.tile([P, NCHUNKS], f32, tag=f"u_cols{b}",
                                name=f"u_cols{b}")
                u_cols.append(uc)

            # ---------------- main loop (2-chunk blocks) ----------------
            def matmuls(j, half):
                if half == 0:
                    ppfs[j] = ps_pf.tile([P, H], f32, name="ppf")
                for q in range(4):
                    fc = half * 4 + q
                    nc.tensor.matmul(
                        ppfs[j][:], featTs[j][:, fc, :], wfT[:, fc, :],
                        start=(fc == 0), stop=(fc == FCHUNKS - 1))

            def epilogue_block(j0):
                last = j0 + 2 >= TOTAL
                b, i0 = divmod(j0, NCHUNKS)
                hidpre2 = eltw.tile([P, 2, H], f32, tag="hidpre2",
                                    name="hidpre2")
                for k, j in enumerate((j0, j0 + 1)):
                    b, i = divmod(j, NCHUNKS)
                    ppf = ppfs[j]
                    pf_sb = eltw.tile([P, H], f32, tag="pf_sb", name="pf_sb")
                    nc.scalar.copy(pf_sb[:], ppf[:])
                    # stores ride the GpSimd queue; loads own the SP queue
                    nc.gpsimd.dma_start(pf_d[b, i * P:(i + 1) * P, :], pf_sb[:])
                    nc.vector.tensor_add(hidpre2[:, k, :], ppf[:], pq_bc[b][:])
                    ftiles[j] = None
                    featTs[j] = None
                    ppfs[j] = None
                # hidden = tanh(pf + pq); u[:, i] = sum_h hid * v — one
                # double-width pass per block
                hid2 = eltw.tile([P, 2, H], f32, tag="hid2", name="hid2")
                prod2 = eltw.tile([P, 2, H], f32, tag="prod2", name="prod2")
                if last:
                    # narrow per-chunk ops shorten the final serial chain
                    for k in range(2):
                        eng = nc.scalar if k == 0 else None
                        nc.scalar.activation(
                            hid2[:, k, :], hidpre2[:, k, :],
                            mybir.ActivationFunctionType.Tanh)
                        nc.vector.tensor_mul(
                            prod2[:, k, :], hid2[:, k, :], v_bc2[:, k, :])
                        nc.vector.reduce_sum(
                            u_cols[b][:, i0 + k:i0 + k + 1], prod2[:, k, :],
                            mybir.AxisListType.X)
                else:
                    nc.scalar.activation(
                        hid2[:], hidpre2[:], mybir.ActivationFunctionType.Tanh)
                    nc.vector.tensor_mul(prod2[:], hid2[:], v_bc2[:])
                    nc.vector.reduce_sum(
                        u_cols[b][:, i0:i0 + 1], prod2[:, 0, :],
                        mybir.AxisListType.X)
                    scr = eltw.tile([P, H], f32, tag="scr", name="scr")
                    nc.scalar.activation(
                        scr[:], prod2[:, 1, :],
                        mybir.ActivationFunctionType.Copy,
                        accum_out=u_cols[b][:, i0 + 1:i0 + 2])

            def batch_tail(b):
                ps_u = ps_t.tile([P, 512], f32, tag="pst", name="ps_u")
                nc.tensor.transpose(ps_u[:NCHUNKS, :P], u_cols[b][:], ident[:])
                lsb = setup.tile([NCHUNKS, P], f32, tag=f"lsb{b}",
                                 name=f"lsb{b}")
                nc.scalar.activation(
                    lsb[:], ps_u[:NCHUNKS, :P],
                    mybir.ActivationFunctionType.Tanh)
                nc.vector.tensor_scalar_mul(lsb[:], lsb[:], TANH_CLIP)
                nc.sync.dma_start(
                    logits_d[b, 0, :].rearrange("(c p) -> c p", p=P), lsb[:])

            for j0 in range(0, TOTAL, 2):
                for j in (j0 + 8, j0 + 9):
                    if j < TOTAL:
                        load(j)
                for j in (j0 + 2, j0 + 3):
                    if j < TOTAL:
                        transposes(j, 0)
                        transposes(j, 1)
                for j in (j0, j0 + 1):
                    matmuls(j, 0)
                    matmuls(j, 1)
                epilogue_block(j0)
                if j0 == NCHUNKS + 2:
                    batch_tail(0)
                if j0 + 2 == TOTAL:
                    batch_tail(BL - 1)

    if split_waits:
        _split_excess_waits(nc)
    return nc


_program = None


def _get_program():
    global _program
    if _program is None:
        _program = build_program()
    return _program


def kernel(features, query, Wf, Wq, v, _trace=False, _trace_kwargs=None):
    nc = _get_program()
    in_maps = []
    for c in range(NCORES):
        sl = slice(c * BL, (c + 1) * BL)
        in_maps.append({
            "features": np.ascontiguousarray(features[sl], dtype=np.float32),
            "query": np.ascontiguousarray(query[sl], dtype=np.float32),
            "Wf": np.ascontiguousarray(Wf, dtype=np.float32),
            "Wq": np.ascontiguousarray(Wq, dtype=np.float32),
            "v": np.ascontiguousarray(v, dtype=np.float32),
        })
    res = run_bass_kernel_spmd(
        nc, in_maps, core_ids=list(range(NCORES)),
        trace=_trace, **(_trace_kwargs or {}))
    pf = np.concatenate([res.results[c]["pf"] for c in range(NCORES)], axis=0)
    logits = np.concatenate(
        [res.results[c]["logits"] for c in range(NCORES)], axis=0)
    if _trace:
        return (pf, logits), res
    return (pf, logits)


# revision 34
# speedup vs baseline: 1.0043x; 1.0043x over previous
"""Trainium2 Bass kernel for nn_Attention_21406117003362.

Computes, for features [B,N,2H], query [B,1,H], Wf [H,2H], Wq [H,H], v [1,1,H]:
  pf     = features @ Wf.T                      [B,N,H]
  pq     = query @ Wq.T                         [B,1,H]
  u      = sum_h v[h] * tanh(pf + pq)           [B,1,N]
  logits = 10 * tanh(u)                         [B,1,N]
returns (pf, logits).

Sharding: data-parallel over B across 8 NeuronCores (2 batches/core),
Wf/Wq/v replicated.

Per-core schedule (single SPMD program):
  - Wf/Wq are PE-transposed once into [f_part, h_free] layout; pq and v are
    broadcast across partitions via tiny K=1 matmuls.
  - Main loop over 64 n-chunks of 128 rows, software-pipelined by one chunk:
    the PE stream interleaves chunk i+1's feature-tile transposes (fp32,
    exact) with chunk i's 8 accumulating float32r (tf32) matmuls so weight
    loads hide under matmul streaming. psum [n128, h512] is the natural
    output layout and is DMA'd straight to DRAM.
  - DVE adds pq; ACT applies tanh and then folds the v-weighted reduction
    over h into one Copy-with-scale + accum_out pass.
  - Per batch, the 32 u-columns are PE-transposed and 10*tanh applied.
"""

import numpy as np

import bass_rust
import concourse.bass as bass
import concourse.mybir as mybir
import concourse.tile as tile
from concourse.bass_utils import run_bass_kernel_spmd
from concourse.masks import make_identity

B, N, H = 16, 4096, 512
F = 2 * H  # 1024
NCORES = 8
BL = B // NCORES  # batches per core = 2
P = 128
NCHUNKS = N // P  # 32 chunks per batch
FCHUNKS = F // P  # 8
TANH_CLIP = 10.0

f32 = mybir.dt.float32
f32r = mybir.dt.float32r

_MAX_WAITS = 1


def _split_excess_waits(nc):
    """This walrus build only encodes 1 sem-wait per instruction; hoist
    extras onto preceding NOPs on the same engine."""
    for bb in nc.m.functions[0].blocks:
        new, changed = [], False
        for inst in bb.instructions:
            si = inst.sync_info
            if si and si.on_wait and len(si.on_wait) > _MAX_WAITS:
                w, k = list(si.on_wait), 0
                while len(w) > _MAX_WAITS:
                    chunk, w = w[:_MAX_WAITS], w[_MAX_WAITS:]
                    new.append(mybir.InstNoOp(
                        name=f"{inst.name}-wsplit{k}", engine=inst.engine,
                        ins=[], outs=[],
                        sync_info=bass_rust.SyncInfo(on_wait=chunk, on_update=[])))
                    k += 1
                inst.sync_info = bass_rust.SyncInfo(on_wait=w, on_update=si.on_update)
                changed = True
            new.append(inst)
        if changed:
            bb.instructions = new


def build_program(split_waits=True):
    nc = bass.Bass("TRN2", target_bir_lowering=False, debug=False)

    feat_d = nc.dram_tensor("features", [BL, N, F], f32, kind="ExternalInput").ap()
    query_d = nc.dram_tensor("query", [BL, 1, H], f32, kind="ExternalInput").ap()
    wf_d = nc.dram_tensor("Wf", [H, F], f32, kind="ExternalInput").ap()
    wq_d = nc.dram_tensor("Wq", [H, H], f32, kind="ExternalInput").ap()
    v_d = nc.dram_tensor("v", [1, 1, H], f32, kind="ExternalInput").ap()

    pf_d = nc.dram_tensor("pf", [BL, N, H], f32, kind="ExternalOutput").ap()
    logits_d = nc.dram_tensor("logits", [BL, 1, N], f32, kind="ExternalOutput").ap()

    HCHUNKS = H // P  # 4
    TOTAL = BL * NCHUNKS  # 64 chunks

    with tile.TileContext(nc) as tc:
        with (
            tc.tile_pool(name="const", bufs=1) as const,
            tc.tile_pool(name="setup", bufs=1) as setup,
            tc.tile_pool(name="feat", bufs=8) as featp,
            tc.tile_pool(name="featT", bufs=6) as featTp,
            tc.tile_pool(name="eltw", bufs=6) as eltw,
            tc.tile_pool(name="ps_t", bufs=5, space="PSUM") as ps_t,
            tc.tile_pool(name="ps_pf", bufs=3, space="PSUM") as ps_pf,
        ):
            # ---------------- warmup + early loads ----------------
            # keep the PE busy from t=0 so the HAM clock-gate reaches 8/8
            # before the real work, and queue feature loads ahead of the
            # weight-setup DMAs on the SP queue
            ident = const.tile([P, P], f32)
            warm = const.tile([P, P], f32)
            nc.vector.memset(warm[:], 0.0)
            make_identity(nc, ident[:])

            ones_row = const.tile([1, P], f32)
            nc.vector.memset(ones_row[:], 1.0)

            HCHUNKS = H // P  # 4
            GCH = H // P  # 4

            ftiles = [None] * TOTAL
            featTs = [None] * TOTAL
            ppfs = [None] * TOTAL

            def load(j):
                b, i = divmod(j, NCHUNKS)
                t = featp.tile([P, F], f32, name="ftile")
                nc.sync.dma_start(t[:], feat_d[b, i * P:(i + 1) * P, :])
                ftiles[j] = t

            load(0)
            load(1)
            wf_sb = setup.tile([P, HCHUNKS, F], f32)
            nc.sync.dma_start(wf_sb[:], wf_d.rearrange("(ho p) f -> p ho f", p=P))
            load(2)
            load(3)

            ps_warm = ps_t.tile([P, 512], f32, tag="pst", name="ps_warm")
            for _ in range(30):
                nc.tensor.matmul(ps_warm[:, :P], warm[:], warm[:],
                                 start=True, stop=True)

            # ---------------- setup ----------------
            load(4)
            load(5)
            wq_sb = setup.tile([P, GCH, H], f32)
            nc.sync.dma_start(wq_sb[:], wq_d.rearrange("(ho p) g -> p ho g", p=P))
            q_sb = setup.tile([BL, H], f32)
            nc.sync.dma_start(q_sb[:], query_d[:, 0, :])
            v_row = setup.tile([1, H], f32)
            nc.sync.dma_start(v_row[:], v_d[0, :, :])
            load(6)
            load(7)

            def transposes(j, half):
                pst = ps_t.tile([P, 512], f32, tag="pst", name="pst")
                for q in range(4):
                    fc = half * 4 + q
                    nc.tensor.transpose(
                        pst[:, q * P:(q + 1) * P],
                        ftiles[j][:, fc * P:(fc + 1) * P],
                        ident[:])
                if featTs[j] is None:
                    featTs[j] = featTp.tile([P, FCHUNKS, P], mybir.dt.float16, name="featT")
                if half == 0:
                    nc.scalar.copy(featTs[j][:, :4, :], pst[:])
                else:
                    nc.vector.tensor_copy(out=featTs[j][:, 4:, :], in_=pst[:])

            # first feature-tile transposes run while Wf/Wq still stream in
            for j in range(2):
                transposes(j, 0)
                transposes(j, 1)

            wfT = const.tile([P, FCHUNKS, H], mybir.dt.float16)
            for fc in range(FCHUNKS):
                pst = ps_t.tile([P, 512], f32, tag="pst", name="pst")
                for ho in range(HCHUNKS):
                    nc.tensor.transpose(
                        pst[:, ho * P:(ho + 1) * P],
                        wf_sb[:, ho, fc * P:(fc + 1) * P], ident[:])
                nc.any.tensor_copy(out=wfT[:, fc, :], in_=pst[:])

            # Wq -> wqT [g_part, gchunk, h]
            wqT = setup.tile([P, GCH, H], f32)
            for gc in range(GCH):
                pst = ps_t.tile([P, 512], f32, tag="pst", name="pst")
                for ho in range(GCH):
                    nc.tensor.transpose(
                        pst[:, ho * P:(ho + 1) * P],
                        wq_sb[:, ho, gc * P:(gc + 1) * P], ident[:])
                nc.any.tensor_copy(out=wqT[:, gc, :], in_=pst[:])

            # query -> qT [g_part, gchunk, b] via PE transpose of [2, 128] rows
            qT = setup.tile([P, GCH, BL], f32)
            for gc in range(GCH):
                pst = ps_t.tile([P, 512], f32, tag="pst", name="pst")
                nc.tensor.transpose(
                    pst[:P, :BL], q_sb[:, gc * P:(gc + 1) * P],
                    ident[:BL, :BL])
                nc.any.tensor_copy(out=qT[:, gc, :], in_=pst[:P, :BL])

            # pq_row_b [1, h] = query[b] @ Wq.T
            pq_rows = []
            for b in range(BL):
                ps_pq = ps_t.tile([P, 512], f32, tag="pst", name="ps_pq")
                for gc in range(GCH):
                    nc.tensor.matmul(
                        ps_pq[:1, :H], qT[:, gc, b:b + 1],
                        wqT[:, gc, :],
                        start=(gc == 0), stop=(gc == GCH - 1))
                t = setup.tile([1, H], f32, tag=f"pq_row{b}")
                nc.any.tensor_copy(out=t[:], in_=ps_pq[:1, :H])
                pq_rows.append(t)

            # broadcast pq rows and v across 128 partitions via K=1 matmuls
            v_bc2 = const.tile([P, 2, H], f32)
            ps_b = ps_t.tile([P, 512], f32, tag="pst", name="ps_b")
            nc.tensor.matmul(ps_b[:, :H], ones_row[:], v_row[:],
                             start=True, stop=True)
            nc.any.tensor_copy(out=v_bc2[:, 0, :], in_=ps_b[:, :H])
            nc.any.tensor_copy(out=v_bc2[:, 1, :], in_=ps_b[:, :H])

            pq_bc = []
            for b in range(BL):
                ps_b2 = ps_t.tile([P, 512], f32, tag="pst", name="ps_b2")
                nc.tensor.matmul(ps_b2[:, :H], ones_row[:], pq_rows[b][:],
                                 start=True, stop=True)
                t = const.tile([P, H], f32, tag=f"pq_bc{b}")
                nc.any.tensor_copy(out=t[:], in_=ps_b2[:, :H])
                pq_bc.append(t)

            u_cols = []
            for b in range(BL):
                uc = const.tile([P, NCHUNKS], f32, tag=f"u_cols{b}",
                                name=f"u_cols{b}")
                u_cols.append(uc)

            # ---------------- main loop (2-chunk blocks) ----------------
            def matmuls(j, half):
                if half == 0:
                    ppfs[j] = ps_pf.tile([P, H], f32, name="ppf")
                for q in range(4):
                    fc = half * 4 + q
                    nc.tensor.matmul(
                        ppfs[j][:], featTs[j][:, fc, :], wfT[:, fc, :],
                        start=(fc == 0), stop=(fc == FCHUNKS - 1))

            def epilogue_block(j0):
                last = j0 + 2 >= TOTAL
                b, i0 = divmod(j0, NCHUNKS)
                hidpre2 = eltw.tile([P, 2, H], f32, tag="hidpre2",
                                    name="hidpre2")
                for k, j in enumerate((j0, j0 + 1)):
                    b, i = divmod(j, NCHUNKS)
                    ppf = ppfs[j]
                    pf_sb = eltw.tile([P, H], f32, tag="pf_sb", name="pf_sb")
                    nc.scalar.copy(pf_sb[:], ppf[:])
                    # stores ride the GpSimd queue; loads own the SP queue
                    nc.gpsimd.dma_start(pf_d[b, i * P:(i + 1) * P, :], pf_sb[:])
                    nc.vector.tensor_add(hidpre2[:, k, :], ppf[:], pq_bc[b][:])
                    ftiles[j] = None
                    featTs[j] = None
                    ppfs[j] = None
                # hidden = tanh(pf + pq); u[:, i] = sum_h hid * v — one
                # double-width pass per block
                hid2 = eltw.tile([P, 2, H], f32, tag="hid2", name="hid2")
                prod2 = eltw.tile([P, 2, H], f32, tag="prod2", name="prod2")
                if last:
                    # narrow per-chunk ops shorten the final serial chain
                    for k in range(2):
                        eng = nc.scalar if k == 0 else None
                        nc.scalar.activation(
                            hid2[:, k, :], hidpre2[:, k, :],
                            mybir.ActivationFunctionType.Tanh)
                        nc.vector.tensor_mul(
                            prod2[:, k, :], hid2[:, k, :], v_bc2[:, k, :])
                        nc.vector.reduce_sum(
                            u_cols[b][:, i0 + k:i0 + k + 1], prod2[:, k, :],
                            mybir.AxisListType.X)
                else:
                    nc.scalar.activation(
                        hid2[:], hidpre2[:], mybir.ActivationFunctionType.Tanh)
                    nc.vector.tensor_mul(prod2[:], hid2[:], v_bc2[:])
                    nc.vector.reduce_sum(
                        u_cols[b][:, i0:i0 + 1], prod2[:, 0, :],
                        mybir.AxisListType.X)
                    scr = eltw.tile([P, H], f32, tag="scr", name="scr")
                    nc.scalar.activation(
                        scr[:], prod2[:, 1, :],
                        mybir.ActivationFunctionType.Copy,
                        accum_out=u_cols[b][:, i0 + 1:i0 + 2])

            def batch_tail(b):
                ps_u = ps_t.tile([P, 512], f32, tag="pst", name="ps_u")
                nc.tensor.transpose(ps_u[:NCHUNKS, :P], u_cols[b][:], ident[:])
                lsb = setup.tile([NCHUNKS, P], f32, tag=f"lsb{b}",
                                 name=f"lsb{b}")
                nc.scalar.activation(
                    lsb[:], ps_u[:NCHUNKS, :P],
                    mybir.ActivationFunctionType.Tanh)
                nc.vector.tensor_scalar_mul(lsb[:], lsb[:], TANH_CLIP)
                nc.sync.dma_start(
                    logits_d[b, 0, :].rearrange("(c p) -> c p", p=P), lsb[:])

            for j0 in range(0, TOTAL, 2):
                for j in (j0 + 8, j0 + 9):
                    if j < TOTAL:
                        load(j)
                for j in (j0 + 2, j0 + 3):
                    if j < TOTAL:
                        transposes(j, 0)
                        transposes(j, 1)
                for j in (j0, j0 + 1):
                    matmuls(j, 0)
                    matmuls(j, 1)
                epilogue_block(j0)
                if j0 == NCHUNKS + 2:
                    batch_tail(0)
                if j0 + 2 == TOTAL:
                    batch_tail(BL - 1)

    if split_waits:
        _split_excess_waits(nc)
    return nc


_program = None


def _get_program():
    global _program
    if _program is None:
        _program = build_program()
    return _program


def kernel(features, query, Wf, Wq, v, _trace=False, _trace_kwargs=None):
    nc = _get_program()
    in_maps = []
    for c in range(NCORES):
        sl = slice(c * BL, (c + 1) * BL)
        in_maps.append({
            "features": np.ascontiguousarray(features[sl], dtype=np.float32),
            "query": np.ascontiguousarray(query[sl], dtype=np.float32),
            "Wf": np.ascontiguousarray(Wf, dtype=np.float32),
            "Wq": np.ascontiguousarray(Wq, dtype=np.float32),
            "v": np.ascontiguousarray(v, dtype=np.float32),
        })
    res = run_bass_kernel_spmd(
        nc, in_maps, core_ids=list(range(NCORES)),
        trace=_trace, **(_trace_kwargs or {}))
    pf = np.concatenate([res.results[c]["pf"] for c in range(NCORES)], axis=0)
    logits = np.concatenate(
        [res.results[c]["logits"] for c in range(NCORES)], axis=0)
    if _trace:
        return (pf, logits), res
    return (pf, logits)


# revision 35
# speedup vs baseline: 1.0126x; 1.0083x over previous
"""Trainium2 Bass kernel for nn_Attention_21406117003362.

Computes, for features [B,N,2H], query [B,1,H], Wf [H,2H], Wq [H,H], v [1,1,H]:
  pf     = features @ Wf.T                      [B,N,H]
  pq     = query @ Wq.T                         [B,1,H]
  u      = sum_h v[h] * tanh(pf + pq)           [B,1,N]
  logits = 10 * tanh(u)                         [B,1,N]
returns (pf, logits).

Sharding: data-parallel over B across 8 NeuronCores (2 batches/core),
Wf/Wq/v replicated.

Per-core schedule (single SPMD program):
  - A short PE warmup covers the Tile preamble + first DMA ramp (HAM stays
    at 8/8); feature loads are queued ahead of the weight-setup DMAs.
  - Wf/Wq are PE-transposed once into [f_part, h_free] layout and cast to
    fp16 (10-bit mantissa, same precision as tf32); pq and v are broadcast
    across partitions via tiny K=1 matmuls.
  - Main loop over 64 n-chunks of 128 rows in 2-chunk blocks: long PE runs
    of fp32 transposes (exact) then fp16 matmuls (FWL hides weight loads),
    one block of lookahead; psum [n128, h512] is the natural output layout.
  - Copybacks cast psum to fp16 split across ACT/DVE; DVE adds pq; ACT
    tanh; the v-weighted reduce over h is split DVE reduce / ACT accum.
  - pf stores ride the GpSimd DMA queue, loads own the SP queue.
  - Per batch, the 32 u-columns are PE-transposed and 10*tanh applied.
"""

import numpy as np

import bass_rust
import concourse.bass as bass
import concourse.mybir as mybir
import concourse.tile as tile
from concourse.bass_utils import run_bass_kernel_spmd
from concourse.masks import make_identity

B, N, H = 16, 4096, 512
F = 2 * H  # 1024
NCORES = 8
BL = B // NCORES  # batches per core = 2
P = 128
NCHUNKS = N // P  # 32 chunks per batch
FCHUNKS = F // P  # 8
TANH_CLIP = 10.0

f32 = mybir.dt.float32
f32r = mybir.dt.float32r

_MAX_WAITS = 1


def _split_excess_waits(nc):
    """This walrus build only encodes 1 sem-wait per instruction; hoist
    extras onto preceding NOPs on the same engine."""
    for bb in nc.m.functions[0].blocks:
        new, changed = [], False
        for inst in bb.instructions:
            si = inst.sync_info
            if si and si.on_wait and len(si.on_wait) > _MAX_WAITS:
                w, k = list(si.on_wait), 0
                while len(w) > _MAX_WAITS:
                    chunk, w = w[:_MAX_WAITS], w[_MAX_WAITS:]
                    new.append(mybir.InstNoOp(
                        name=f"{inst.name}-wsplit{k}", engine=inst.engine,
                        ins=[], outs=[],
                        sync_info=bass_rust.SyncInfo(on_wait=chunk, on_update=[])))
                    k += 1
                inst.sync_info = bass_rust.SyncInfo(on_wait=w, on_update=si.on_update)
                changed = True
            new.append(inst)
        if changed:
            bb.instructions = new


def build_program(split_waits=True):
    nc = bass.Bass("TRN2", target_bir_lowering=False, debug=False)

    feat_d = nc.dram_tensor("features", [BL, N, F], f32, kind="ExternalInput").ap()
    query_d = nc.dram_tensor("query", [BL, 1, H], f32, kind="ExternalInput").ap()
    wf_d = nc.dram_tensor("Wf", [H, F], f32, kind="ExternalInput").ap()
    wq_d = nc.dram_tensor("Wq", [H, H], f32, kind="ExternalInput").ap()
    v_d = nc.dram_tensor("v", [1, 1, H], f32, kind="ExternalInput").ap()

    pf_d = nc.dram_tensor("pf", [BL, N, H], f32, kind="ExternalOutput").ap()
    logits_d = nc.dram_tensor("logits", [BL, 1, N], f32, kind="ExternalOutput").ap()

    HCHUNKS = H // P  # 4
    TOTAL = BL * NCHUNKS  # 64 chunks

    with tile.TileContext(nc) as tc:
        with (
            tc.tile_pool(name="const", bufs=1) as const,
            tc.tile_pool(name="setup", bufs=1) as setup,
            tc.tile_pool(name="feat", bufs=8) as featp,
            tc.tile_pool(name="featT", bufs=6) as featTp,
            tc.tile_pool(name="eltw", bufs=6) as eltw,
            tc.tile_pool(name="ps_t", bufs=5, space="PSUM") as ps_t,
            tc.tile_pool(name="ps_pf", bufs=3, space="PSUM") as ps_pf,
        ):
            # ---------------- warmup + early loads ----------------
            # keep the PE busy from t=0 so the HAM clock-gate reaches 8/8
            # before the real work, and queue feature loads ahead of the
            # weight-setup DMAs on the SP queue
            ident = const.tile([P, P], f32)
            warm = const.tile([P, P], f32)
            nc.vector.memset(warm[:], 0.0)
            make_identity(nc, ident[:])

            ones_row = const.tile([1, P], f32)
            nc.vector.memset(ones_row[:], 1.0)

            HCHUNKS = H // P  # 4
            GCH = H // P  # 4

            ftiles = [None] * TOTAL
            featTs = [None] * TOTAL
            ppfs = [None] * TOTAL

            def load(j):
                b, i = divmod(j, NCHUNKS)
                t = featp.tile([P, F], f32, name="ftile")
                nc.sync.dma_start(t[:], feat_d[b, i * P:(i + 1) * P, :])
                ftiles[j] = t

            load(0)
            load(1)
            wf_sb = setup.tile([P, HCHUNKS, F], f32)
            nc.sync.dma_start(wf_sb[:], wf_d.rearrange("(ho p) f -> p ho f", p=P))
            load(2)
            load(3)

            ps_warm = ps_t.tile([P, 512], f32, tag="pst", name="ps_warm")
            for _ in range(30):
                nc.tensor.matmul(ps_warm[:, :P], warm[:], warm[:],
                                 start=True, stop=True)

            # ---------------- setup ----------------
            load(4)
            load(5)
            wq_sb = setup.tile([P, GCH, H], f32)
            nc.sync.dma_start(wq_sb[:], wq_d.rearrange("(ho p) g -> p ho g", p=P))
            q_sb = setup.tile([BL, H], f32)
            nc.sync.dma_start(q_sb[:], query_d[:, 0, :])
            v_row = setup.tile([1, H], f32)
            nc.sync.dma_start(v_row[:], v_d[0, :, :])
            load(6)
            load(7)

            def transposes(j, half):
                pst = ps_t.tile([P, 512], f32, tag="pst", name="pst")
                for q in range(4):
                    fc = half * 4 + q
                    nc.tensor.transpose(
                        pst[:, q * P:(q + 1) * P],
                        ftiles[j][:, fc * P:(fc + 1) * P],
                        ident[:])
                if featTs[j] is None:
                    featTs[j] = featTp.tile([P, FCHUNKS, P], mybir.dt.float16, name="featT")
                if half == 0:
                    nc.scalar.copy(featTs[j][:, :4, :], pst[:])
                else:
                    nc.vector.tensor_copy(out=featTs[j][:, 4:, :], in_=pst[:])

            # first feature-tile transposes run while Wf/Wq still stream in
            for j in range(2):
                transposes(j, 0)
                transposes(j, 1)

            wfT = const.tile([P, FCHUNKS, H], mybir.dt.float16)
            for fc in range(FCHUNKS):
                pst = ps_t.tile([P, 512], f32, tag="pst", name="pst")
                for ho in range(HCHUNKS):
                    nc.tensor.transpose(
                        pst[:, ho * P:(ho + 1) * P],
                        wf_sb[:, ho, fc * P:(fc + 1) * P], ident[:])
                nc.any.tensor_copy(out=wfT[:, fc, :], in_=pst[:])

            # Wq -> wqT [g_part, gchunk, h]
            wqT = setup.tile([P, GCH, H], f32)
            for gc in range(GCH):
                pst = ps_t.tile([P, 512], f32, tag="pst", name="pst")
                for ho in range(GCH):
                    nc.tensor.transpose(
                        pst[:, ho * P:(ho + 1) * P],
                        wq_sb[:, ho, gc * P:(gc + 1) * P], ident[:])
                nc.any.tensor_copy(out=wqT[:, gc, :], in_=pst[:])

            # query -> qT [g_part, gchunk, b] via PE transpose of [2, 128] rows
            qT = setup.tile([P, GCH, BL], f32)
            for gc in range(GCH):
                pst = ps_t.tile([P, 512], f32, tag="pst", name="pst")
                nc.tensor.transpose(
                    pst[:P, :BL], q_sb[:, gc * P:(gc + 1) * P],
                    ident[:BL, :BL])
                nc.any.tensor_copy(out=qT[:, gc, :], in_=pst[:P, :BL])

            # pq_row_b [1, h] = query[b] @ Wq.T
            pq_rows = []
            for b in range(BL):
                ps_pq = ps_t.tile([P, 512], f32, tag="pst", name="ps_pq")
                for gc in range(GCH):
                    nc.tensor.matmul(
                        ps_pq[:1, :H], qT[:, gc, b:b + 1],
                        wqT[:, gc, :],
                        start=(gc == 0), stop=(gc == GCH - 1))
                t = setup.tile([1, H], f32, tag=f"pq_row{b}")
                nc.any.tensor_copy(out=t[:], in_=ps_pq[:1, :H])
                pq_rows.append(t)

            # broadcast pq rows and v across 128 partitions via K=1 matmuls
            v_bc2 = const.tile([P, 2, H], f32)
            ps_b = ps_t.tile([P, 512], f32, tag="pst", name="ps_b")
            nc.tensor.matmul(ps_b[:, :H], ones_row[:], v_row[:],
                             start=True, stop=True)
            nc.any.tensor_copy(out=v_bc2[:, 0, :], in_=ps_b[:, :H])
            nc.any.tensor_copy(out=v_bc2[:, 1, :], in_=ps_b[:, :H])

            pq_bc = []
            for b in range(BL):
                ps_b2 = ps_t.tile([P, 512], f32, tag="pst", name="ps_b2")
                nc.tensor.matmul(ps_b2[:, :H], ones_row[:], pq_rows[b][:],
                                 start=True, stop=True)
                t = const.tile([P, H], f32, tag=f"pq_bc{b}")
                nc.any.tensor_copy(out=t[:], in_=ps_b2[:, :H])
                pq_bc.append(t)

            u_cols = []
            for b in range(BL):
                uc = const.tile([P, NCHUNKS], f32, tag=f"u_cols{b}",
                                name=f"u_cols{b}")
                u_cols.append(uc)

            # ---------------- main loop (2-chunk blocks) ----------------
            def matmuls(j, half):
                if half == 0:
                    ppfs[j] = ps_pf.tile([P, H], f32, name="ppf")
                for q in range(4):
                    fc = half * 4 + q
                    nc.tensor.matmul(
                        ppfs[j][:], featTs[j][:, fc, :], wfT[:, fc, :],
                        start=(fc == 0), stop=(fc == FCHUNKS - 1))

            def epilogue_block(j0):
                last = j0 + 2 >= TOTAL
                b, i0 = divmod(j0, NCHUNKS)
                hidpre2 = eltw.tile([P, 2, H], f32, tag="hidpre2",
                                    name="hidpre2")
                for k, j in enumerate((j0, j0 + 1)):
                    b, i = divmod(j, NCHUNKS)
                    ppf = ppfs[j]
                    pf_sb = eltw.tile([P, H], f32, tag="pf_sb", name="pf_sb")
                    nc.scalar.copy(pf_sb[:], ppf[:])
                    # stores ride the GpSimd queue; loads own the SP queue
                    nc.gpsimd.dma_start(pf_d[b, i * P:(i + 1) * P, :], pf_sb[:])
                    nc.vector.tensor_add(hidpre2[:, k, :], ppf[:], pq_bc[b][:])
                    ftiles[j] = None
                    featTs[j] = None
                    ppfs[j] = None
                # hidden = tanh(pf + pq); u[:, i] = sum_h hid * v — one
                # double-width pass per block
                hid2 = eltw.tile([P, 2, H], f32, tag="hid2", name="hid2")
                prod2 = eltw.tile([P, 2, H], f32, tag="prod2", name="prod2")
                if last:
                    # narrow per-chunk ops shorten the final serial chain
                    for k in range(2):
                        eng = nc.scalar if k == 0 else None
                        nc.scalar.activation(
                            hid2[:, k, :], hidpre2[:, k, :],
                            mybir.ActivationFunctionType.Tanh)
                        nc.vector.tensor_mul(
                            prod2[:, k, :], hid2[:, k, :], v_bc2[:, k, :])
                        nc.vector.reduce_sum(
                            u_cols[b][:, i0 + k:i0 + k + 1], prod2[:, k, :],
                            mybir.AxisListType.X)
                else:
                    nc.scalar.activation(
                        hid2[:], hidpre2[:], mybir.ActivationFunctionType.Tanh)
                    nc.vector.tensor_mul(prod2[:], hid2[:], v_bc2[:])
                    nc.vector.reduce_sum(
                        u_cols[b][:, i0:i0 + 1], prod2[:, 0, :],
                        mybir.AxisListType.X)
                    scr = eltw.tile([P, H], f32, tag="scr", name="scr")
                    nc.scalar.activation(
                        scr[:], prod2[:, 1, :],
                        mybir.ActivationFunctionType.Copy,
                        accum_out=u_cols[b][:, i0 + 1:i0 + 2])

            def batch_tail(b):
                ps_u = ps_t.tile([P, 512], f32, tag="pst", name="ps_u")
                nc.tensor.transpose(ps_u[:NCHUNKS, :P], u_cols[b][:], ident[:])
                lsb = setup.tile([NCHUNKS, P], f32, tag=f"lsb{b}",
                                 name=f"lsb{b}")
                nc.scalar.activation(
                    lsb[:], ps_u[:NCHUNKS, :P],
                    mybir.ActivationFunctionType.Tanh)
                nc.vector.tensor_scalar_mul(lsb[:], lsb[:], TANH_CLIP)
                nc.sync.dma_start(
                    logits_d[b, 0, :].rearrange("(c p) -> c p", p=P), lsb[:])

            for j0 in range(0, TOTAL, 2):
                for j in (j0 + 8, j0 + 9):
                    if j < TOTAL:
                        load(j)
                for j in (j0 + 2, j0 + 3):
                    if j < TOTAL:
                        transposes(j, 0)
                        transposes(j, 1)
                for j in (j0, j0 + 1):
                    matmuls(j, 0)
                    matmuls(j, 1)
                epilogue_block(j0)
                if j0 == NCHUNKS + 2:
                    batch_tail(0)
                if j0 + 2 == TOTAL:
                    batch_tail(BL - 1)

    if split_waits:
        _split_excess_waits(nc)
    return nc


_program = None


def _get_program():
    global _program
    if _program is None:
        _program = build_program()
    return _program


def kernel(features, query, Wf, Wq, v, _trace=False, _trace_kwargs=None):
    nc = _get_program()
    in_maps = []
    for c in range(NCORES):
        sl = slice(c * BL, (c + 1) * BL)
        in_maps.append({
            "features": np.ascontiguousarray(features[sl], dtype=np.float32),
            "query": np.ascontiguousarray(query[sl], dtype=np.float32),
            "Wf": np.ascontiguousarray(Wf, dtype=np.float32),
            "Wq": np.ascontiguousarray(Wq, dtype=np.float32),
            "v": np.ascontiguousarray(v, dtype=np.float32),
        })
    res = run_bass_kernel_spmd(
        nc, in_maps, core_ids=list(range(NCORES)),
        trace=_trace, **(_trace_kwargs or {}))
    pf = np.concatenate([res.results[c]["pf"] for c in range(NCORES)], axis=0)
    logits = np.concatenate(
        [res.results[c]["logits"] for c in range(NCORES)], axis=0)
    if _trace:
        return (pf, logits), res
    return (pf, logits)
